# revision 18
# baseline (speedup 1.0000x reference)
"""Self-contained Trainium2 Bass kernel for the DeepseekV2 decoder layer problem.

Sharding (8 cores): each core owns 128 query tokens (64-token low block c +
64-token high block 15-c for causal balance), all 16 heads.  KV-side
projections are computed per-own-token and AllGathered (K^T/kpe^T/ik^T bundle
+ V).  Attention + o_proj run on own rows.  A second AllGather shares h2
(bf16) and exact-f32 router weights; MoE is expert-parallel (1 routed expert
per core, dense over all tokens) plus a 1/8 slice of the shared expert.
Host sums the per-core partials and un-permutes rows.
"""
import sys
sys.path.insert(0, "/opt/trn_rl_repo")
import numpy as np
import ml_dtypes

import concourse.bass as bass
import concourse.mybir as mybir
from concourse import bacc, tile
from concourse.bass_utils import run_bass_kernel_spmd
from concourse.masks import make_identity

f32 = mybir.dt.float32
bf16 = mybir.dt.bfloat16
AF = mybir.ActivationFunctionType
ALU = mybir.AluOpType
AX = mybir.AxisListType
BF = ml_dtypes.bfloat16

# dims
T = 1024; H = 2048; NH = 16; DN = 128; DR = 64; DQ = DN + DR; DV = 128
QL = 1536; KL = 512
INH = 16; IHD = 128; TOPK = 256
NE = 8; MI = 1024; SI = 1024
BASE = 10000.0; EPS = 1e-6
SCALE = DQ ** -0.5
IDX_SCALE = IHD ** -0.5
FP8_MAX = 448.0
NCORES = 8
TPC = T // NCORES        # 128 tokens per core
NEG = -1e30


def own_rows(c):
    lo = list(range(64 * c, 64 * c + 64))
    hi = list(range(T - 64 * c - 64, T - 64 * c))
    return lo + hi


PERM = np.array([r for c in range(NCORES) for r in own_rows(c)], dtype=np.int64)


def build():
    nc = bacc.Bacc("TRN2", target_bir_lowering=False, debug=False,
                   enable_asserts=False, num_devices=NCORES)

    def din(name, shape, dt=bf16):
        return nc.dram_tensor(name, shape, dt, kind="ExternalInput").ap()

    # ---- per-core inputs ----
    XO = din("XO", [TPC, H], f32)              # x_in own rows
    CAUS = din("CAUS", [TPC, T], f32)          # causal01 over global keys
    CSQ = din("CSQ", [TPC, 512], f32)          # cos*SCALE tiled 16x
    SNQ = din("SNQ", [TPC, 512], f32)
    CSR = din("CSR", [TPC, 512], f32)          # cos tiled 16x (unscaled)
    SNR = din("SNR", [TPC, 512], f32)
    OH = din("OH", [8, 1], f32)                # expert one-hot
    KNW = din("KNW", [1, IHD], f32)            # idx_kn_w
    KNB = din("KNB", [1, IHD], f32)
    WPB = din("WPB", [1, INH], f32)            # idx_wp_b
    WA = din("WA", [H, QL + KL + DR])          # bf16, ln-folded
    WQB = din("WQB", [QL, NH * DQ])
    WIQ = din("WIQ", [QL, INH * IHD])
    WIK = din("WIK", [H, IHD])
    WIP = din("WIP", [H, INH])
    WKN = din("WKN", [KL, NH * DN])
    WV = din("WV", [KL, NH * DV])
    WO = din("WO", [NH * DV, H])
    WG = din("WG", [H, NE], f32)
    WEG = din("WEG", [H, MI])
    WEU = din("WEU", [H, MI])
    WED = din("WED", [MI, H])
    WSG = din("WSG", [H, SI])
    WSU = din("WSU", [H, SI])
    WSD = din("WSD", [SI, H])

    OUT_P = nc.dram_tensor("OUT_P", [T, H], f32, kind="ExternalOutput").ap()
    OUT_X = nc.dram_tensor("OUT_X", [TPC, H], f32, kind="ExternalOutput").ap()

    KB = 16   # H/128 k-chunks
    QB = 12   # QL/128
    RG = [list(range(NCORES))]

    with tile.TileContext(nc) as tc:
        with tc.tile_pool(name="const", bufs=1) as Pc, \
             tc.tile_pool(name="dram", bufs=1, space="DRAM") as Pd:
            idf = Pc.tile([128, 128], f32)
            make_identity(nc, idf[:])
            idb = Pc.tile([128, 128], bf16)
            nc.vector.tensor_copy(idb[:], idf[:])
            eps_b = Pc.tile([128, 1], f32)
            nc.vector.memset(eps_b[:], EPS)

            xo = Pc.tile([TPC, H], f32)
            nc.sync.dma_start(xo[:], XO[:])
            caus = Pc.tile([TPC, T], f32)
            nc.sync.dma_start(caus[:], CAUS[:])
            csq = Pc.tile([TPC, 512], f32); nc.sync.dma_start(csq[:], CSQ[:])
            snq = Pc.tile([TPC, 512], f32); nc.sync.dma_start(snq[:], SNQ[:])
            csr = Pc.tile([TPC, 512], f32); nc.sync.dma_start(csr[:], CSR[:])
            snr = Pc.tile([TPC, 512], f32); nc.sync.dma_start(snr[:], SNR[:])
            oh = Pc.tile([8, 1], f32); nc.sync.dma_start(oh[:], OH[:])
            knw_r = Pc.tile([1, IHD], f32); nc.sync.dma_start(knw_r[:], KNW[:])
            knb_r = Pc.tile([1, IHD], f32); nc.sync.dma_start(knb_r[:], KNB[:])
            wpb_r = Pc.tile([1, INH], f32); nc.sync.dma_start(wpb_r[:], WPB[:])
            knw_bc = Pc.tile([128, IHD], f32)
            nc.gpsimd.partition_broadcast(knw_bc[:], knw_r[:])
            knb_bc = Pc.tile([128, IHD], f32)
            nc.gpsimd.partition_broadcast(knb_bc[:], knb_r[:])
            wpb_bc = Pc.tile([128, INH], f32)
            nc.gpsimd.partition_broadcast(wpb_bc[:], wpb_r[:])
            wg_sb = Pc.tile([128, KB, NE], f32)
            nc.sync.dma_start(wg_sb[:], WG[:].rearrange("(k p) n -> p k n", p=128))

            # collective buffers
            CCB = KL + DR + IHD                                   # 704 rows
            cc1_in = Pd.tile([CCB, TPC], bf16)
            cc1_out = Pd.tile([NCORES, CCB, TPC], bf16, addr_space="Shared")
            cch_in = Pd.tile([TPC, H], bf16)
            cch_out = Pd.tile([NCORES, TPC, H], bf16, addr_space="Shared")
            ccr_in = Pd.tile([TPC, NE], f32)
            ccr_out = Pd.tile([NCORES, TPC, NE], f32, addr_space="Shared")

            with tc.tile_pool(name="att", bufs=1) as Pa, \
                 tc.tile_pool(name="wstream", bufs=2) as Pw:
                # rmsnorm scale r1 for own rows
                sq = Pa.tile([TPC, H], f32, name="sq_scratch", tag="sq2")
                ssq = Pa.tile([TPC, 1], f32)
                nc.scalar.activation(sq[:], xo[:], AF.Square, accum_out=ssq[:])
                r1 = Pa.tile([TPC, 1], f32)
                nc.scalar.activation(r1[:], ssq[:], AF.Sqrt, bias=eps_b[:], scale=1.0 / H)
                nc.vector.reciprocal(r1[:], r1[:])
                # h_normT own chunks: from PE transposes of hn_own
                hn_own = Pa.tile([TPC, H], bf16)
                nc.vector.tensor_scalar(hn_own[:], xo[:], r1[:], None, op0=ALU.mult)
                hnT = Pa.tile([128, KB, TPC], bf16)
                with tc.tile_pool(name="ps_tr", bufs=2, space="PSUM") as Pp:
                    for k in range(KB):
                        tp = Pp.tile([128, 128], bf16, name="tp")
                        nc.tensor.transpose(tp[:], hn_own[:, k * 128:(k + 1) * 128], idb[:])
                        nc.scalar.copy(hnT[:, k, :], tp[:])

                # ---- qkv_a ----
                with tc.tile_pool(name="ps_qkv", bufs=1, space="PSUM") as Pp:
                    qc_ps = Pp.tile([TPC, QL], f32)
                    kv_ps = Pp.tile([TPC, KL + DR], f32)
                    for k in range(KB):
                        wa_k = Pw.tile([128, QL + KL + DR], bf16, name="wstream", tag="wstream")
                        nc.sync.dma_start(wa_k[:], WA[:].rearrange("(k p) n -> p k n", p=128)[:, k, :])
                        for j in range(3):
                            nc.tensor.matmul(qc_ps[:, j * 512:(j + 1) * 512],
                                             hnT[:, k, :], wa_k[:, j * 512:(j + 1) * 512],
                                             start=(k == 0), stop=(k == KB - 1))
                        nc.tensor.matmul(kv_ps[:, 0:512], hnT[:, k, :],
                                         wa_k[:, QL:QL + 512], start=(k == 0), stop=(k == KB - 1))
                        nc.tensor.matmul(kv_ps[:, 512:KL + DR], hnT[:, k, :],
                                         wa_k[:, QL + 512:], start=(k == 0), stop=(k == KB - 1))

                    # q_c rmsnorm -> bf16
                    qsq = Pa.tile([TPC, QL], f32, name="qsq", tag="sq2")
                    qss = Pa.tile([TPC, 1], f32)
                    nc.scalar.activation(qsq[:], qc_ps[:], AF.Square, accum_out=qss[:])
                    rq = Pa.tile([TPC, 1], f32)
                    nc.scalar.activation(rq[:], qss[:], AF.Sqrt, bias=eps_b[:], scale=1.0 / QL)
                    nc.vector.reciprocal(rq[:], rq[:])
                    qcn = Pa.tile([TPC, QL], bf16)
                    nc.vector.tensor_scalar(qcn[:], qc_ps[:], rq[:], None, op0=ALU.mult)

                    # kv_c rmsnorm -> bf16
                    ksq = Pa.tile([TPC, KL], f32, name="ksq", tag="sq2")
                    kss = Pa.tile([TPC, 1], f32)
                    nc.scalar.activation(ksq[:], kv_ps[:, :KL], AF.Square, accum_out=kss[:])
                    rkv = Pa.tile([TPC, 1], f32)
                    nc.scalar.activation(rkv[:], kss[:], AF.Sqrt, bias=eps_b[:], scale=1.0 / KL)
                    nc.vector.reciprocal(rkv[:], rkv[:])
                    kvn = Pa.tile([TPC, KL], bf16)
                    nc.vector.tensor_scalar(kvn[:], kv_ps[:, :KL], rkv[:], None, op0=ALU.mult)

                    # k_pe rope (unscaled tables) -> bf16 [TPC, 64]
                    kpe = Pa.tile([TPC, DR], bf16)
                    t1 = Pa.tile([TPC, 32], f32, name="rt1", tag="rt1")
                    t2 = Pa.tile([TPC, 32], f32, name="rt2", tag="rt2")
                    pe_src = kv_ps[:, KL:].rearrange("p (n two) -> p n two", two=2)
                    x1, x2 = pe_src[:, :, 0], pe_src[:, :, 1]
                    ko = kpe[:].rearrange("p (n two) -> p n two", two=2)
                    nc.vector.tensor_tensor(t1[:], x1, csr[:, :32], op=ALU.mult)
                    nc.vector.tensor_tensor(t2[:], x2, snr[:, :32], op=ALU.mult)
                    nc.vector.tensor_sub(ko[:, :, 0], t1[:], t2[:])
                    nc.vector.tensor_tensor(t1[:], x1, snr[:, :32], op=ALU.mult)
                    nc.vector.tensor_tensor(t2[:], x2, csr[:, :32], op=ALU.mult)
                    nc.vector.tensor_add(ko[:, :, 1], t1[:], t2[:])

                # transposes of qcn, kvn, kpe
                qcT = Pa.tile([128, QB, TPC], bf16)
                kvT = Pa.tile([128, 4, TPC], bf16)
                kpeT = Pa.tile([DR, TPC], bf16)
                with tc.tile_pool(name="ps_tr2", bufs=2, space="PSUM") as Pp:
                    for k in range(QB):
                        tpq = Pp.tile([128, 128], bf16, name="tpq", tag="tp")
                        nc.tensor.transpose(tpq[:], qcn[:, k * 128:(k + 1) * 128], idb[:])
                        nc.scalar.copy(qcT[:, k, :], tpq[:])
                    for k in range(4):
                        tpk = Pp.tile([128, 128], bf16, name="tpk", tag="tp")
                        nc.tensor.transpose(tpk[:], kvn[:, k * 128:(k + 1) * 128], idb[:])
                        nc.scalar.copy(kvT[:, k, :], tpk[:])
                    tpp = Pp.tile([DR, 128], bf16, name="tpp", tag="tp")
                    nc.tensor.transpose(tpp[:], kpe[:], idb[:])
                    nc.scalar.copy(kpeT[:], tpp[:])

                # ---- ik own: layernorm(hn @ Wik) + rope ----
                ikn = Pa.tile([TPC, IHD], bf16)
                iknT_own = Pa.tile([IHD, TPC], bf16)
                with tc.tile_pool(name="ps_ik", bufs=1, space="PSUM") as Pp:
                    wik_sb = Pa.tile([128, KB, IHD], bf16)
                    nc.sync.dma_start(wik_sb[:], WIK[:].rearrange("(k p) n -> p k n", p=128))
                    ik_ps = Pp.tile([TPC, IHD], f32)
                    for k in range(KB):
                        nc.tensor.matmul(ik_ps[:], hnT[:, k, :], wik_sb[:, k, :],
                                         start=(k == 0), stop=(k == KB - 1))
                    negm = Pa.tile([TPC, 1], f32)
                    nc.vector.tensor_reduce(negm[:], ik_ps[:], AX.X, ALU.add, negate=True)
                    nc.vector.tensor_scalar(negm[:], negm[:], 1.0 / IHD, None, op0=ALU.mult)
                    xm = Pa.tile([TPC, IHD], f32)
                    nc.vector.tensor_scalar(xm[:], ik_ps[:], negm[:], None, op0=ALU.add)
                    xms = Pa.tile([TPC, IHD], f32)
                    vss = Pa.tile([TPC, 1], f32)
                    nc.scalar.activation(xms[:], xm[:], AF.Square, accum_out=vss[:])
                    rstd = Pa.tile([TPC, 1], f32)
                    nc.scalar.activation(rstd[:], vss[:], AF.Sqrt, bias=eps_b[:], scale=1.0 / IHD)
                    nc.vector.reciprocal(rstd[:], rstd[:])
                    ikf = Pa.tile([TPC, IHD], f32)
                    nc.vector.scalar_tensor_tensor(ikf[:], xm[:], rstd[:], knw_bc[:],
                                                   op0=ALU.mult, op1=ALU.mult)
                    nc.vector.tensor_add(ikf[:], ikf[:], knb_bc[:])
                    # rope first 64 dims
                    pe2 = ikf[:, :DR].rearrange("p (n two) -> p n two", two=2)
                    iko = ikn[:].rearrange("p d -> p d")
                    iko2 = ikn[:, :DR].rearrange("p (n two) -> p n two", two=2)
                    it1 = Pa.tile([TPC, 32], f32, name="it1", tag="rt1")
                    it2 = Pa.tile([TPC, 32], f32, name="it2", tag="rt2")
                    nc.vector.tensor_tensor(it1[:], pe2[:, :, 0], csr[:, :32], op=ALU.mult)
                    nc.vector.tensor_tensor(it2[:], pe2[:, :, 1], snr[:, :32], op=ALU.mult)
                    nc.vector.tensor_sub(iko2[:, :, 0], it1[:], it2[:])
                    nc.vector.tensor_tensor(it1[:], pe2[:, :, 0], snr[:, :32], op=ALU.mult)
                    nc.vector.tensor_tensor(it2[:], pe2[:, :, 1], csr[:, :32], op=ALU.mult)
                    nc.vector.tensor_add(iko2[:, :, 1], it1[:], it2[:])
                    nc.vector.tensor_copy(ikn[:, DR:], ikf[:, DR:])
                with tc.tile_pool(name="ps_ikt", bufs=1, space="PSUM") as Pp:
                    tpi = Pp.tile([IHD, TPC], bf16)
                    nc.tensor.transpose(tpi[:], ikn[:], idb[:])
                    nc.scalar.copy(iknT_own[:], tpi[:])

                # ---- CC#1: bundle kv_cn^T, kpe^T, ik^T ----
                nc.sync.dma_start(cc1_in[:KL, :].rearrange("(k p) t -> p k t", p=128), kvT[:])
                nc.sync.dma_start(cc1_in[KL:KL + DR, :], kpeT[:])
                nc.sync.dma_start(cc1_in[KL + DR:, :], iknT_own[:])
                nc.gpsimd.collective_compute("AllGather", ALU.bypass, replica_groups=RG,
                                             ins=[cc1_in[:].opt()], outs=[cc1_out[:].opt()])

                # ---- Q / iq / wts (overlaps CC) ----
                qtn = Pa.tile([TPC, NH, DN], bf16)    # q_nope * SCALE
                qtp = Pa.tile([TPC, NH, DR], bf16)    # roped q_pe * SCALE
                with tc.tile_pool(name="ps_q", bufs=1, space="PSUM") as Pp:
                    q_ps = Pp.tile([TPC, NH * DQ], f32)
                    for k in range(QB):
                        wqb_k = Pw.tile([128, NH * DQ], bf16, name="wstream2", tag="wstream")
                        nc.sync.dma_start(wqb_k[:], WQB[:].rearrange("(k p) n -> p k n", p=128)[:, k, :])
                        for j in range(6):
                            nc.tensor.matmul(q_ps[:, j * 512:(j + 1) * 512], qcT[:, k, :],
                                             wqb_k[:, j * 512:(j + 1) * 512],
                                             start=(k == 0), stop=(k == QB - 1))
                    qv = q_ps[:].rearrange("p (h d) -> p h d", h=NH)
                    nc.vector.tensor_scalar(qtn[:], qv[:, :, :DN], SCALE, None, op0=ALU.mult)
                    pe3 = qv[:, :, DN:].rearrange("p h (n two) -> p h n two", two=2)
                    qo3 = qtp[:].rearrange("p h (n two) -> p h n two", two=2)
                    c3 = csq[:].rearrange("p (h n) -> p h n", h=NH)
                    s3 = snq[:].rearrange("p (h n) -> p h n", h=NH)
                    qt1 = Pa.tile([TPC, NH, 32], f32, name="qt1")
                    qt2 = Pa.tile([TPC, NH, 32], f32, name="qt2")
                    nc.vector.tensor_tensor(qt1[:], pe3[:, :, :, 0], c3, op=ALU.mult)
                    nc.vector.tensor_tensor(qt2[:], pe3[:, :, :, 1], s3, op=ALU.mult)
                    nc.vector.tensor_sub(qo3[:, :, :, 0], qt1[:], qt2[:])
                    nc.vector.tensor_tensor(qt1[:], pe3[:, :, :, 0], s3, op=ALU.mult)
                    nc.vector.tensor_tensor(qt2[:], pe3[:, :, :, 1], c3, op=ALU.mult)
                    nc.vector.tensor_add(qo3[:, :, :, 1], qt1[:], qt2[:])

                qtnT = Pa.tile([DN, NH, TPC], bf16)
                qtpT = Pa.tile([DR, NH, TPC], bf16)
                with tc.tile_pool(name="ps_qt", bufs=2, space="PSUM") as Pp:
                    for h in range(NH):
                        tq1 = Pp.tile([DN, TPC], bf16, name="tq1", tag="tp")
                        nc.tensor.transpose(tq1[:], qtn[:, h, :], idb[:])
                        nc.scalar.copy(qtnT[:, h, :], tq1[:])
                        tq2 = Pp.tile([DR, TPC], bf16, name="tq2", tag="tp")
                        nc.tensor.transpose(tq2[:], qtp[:, h, :], idb[:])
                        nc.scalar.copy(qtpT[:, h, :], tq2[:])

                # iq
                iq_bf = Pa.tile([TPC, INH, IHD], bf16)
                qscale = Pa.tile([TPC, INH], f32)
                with tc.tile_pool(name="ps_iq", bufs=1, space="PSUM") as Pp:
                    iq_ps = Pp.tile([TPC, INH * IHD], f32)
                    for k in range(QB):
                        wiq_k = Pw.tile([128, INH * IHD], bf16, name="wstream3", tag="wstream")
                        nc.sync.dma_start(wiq_k[:], WIQ[:].rearrange("(k p) n -> p k n", p=128)[:, k, :])
                        for j in range(4):
                            nc.tensor.matmul(iq_ps[:, j * 512:(j + 1) * 512], qcT[:, k, :],
                                             wiq_k[:, j * 512:(j + 1) * 512],
                                             start=(k == 0), stop=(k == QB - 1))
                    iqv = iq_ps[:].rearrange("p (h d) -> p h d", h=INH)
                    ipe = iqv[:, :, :DR].rearrange("p h (n two) -> p h n two", two=2)
                    ioe = iq_bf[:, :, :DR].rearrange("p h (n two) -> p h n two", two=2)
                    c3r = csr[:].rearrange("p (h n) -> p h n", h=NH)
                    s3r = snr[:].rearrange("p (h n) -> p h n", h=NH)
                    iq1 = Pa.tile([TPC, INH, 32], f32, name="iq1", tag="qt1")
                    iq2 = Pa.tile([TPC, INH, 32], f32, name="iq2", tag="qt2")
                    nc.vector.tensor_tensor(iq1[:], ipe[:, :, :, 0], c3r, op=ALU.mult)
                    nc.vector.tensor_tensor(iq2[:], ipe[:, :, :, 1], s3r, op=ALU.mult)
                    nc.vector.tensor_sub(ioe[:, :, :, 0], iq1[:], iq2[:])
                    nc.vector.tensor_tensor(iq1[:], ipe[:, :, :, 0], s3r, op=ALU.mult)
                    nc.vector.tensor_tensor(iq2[:], ipe[:, :, :, 1], c3r, op=ALU.mult)
                    nc.vector.tensor_add(ioe[:, :, :, 1], iq1[:], iq2[:])
                    nc.vector.tensor_copy(iq_bf[:, :, DR:], iqv[:, :, DR:])
                    # amax over head dims (from bf16 tile -> matches scoring values)
                    nc.vector.tensor_reduce(qscale[:], iq_bf[:], AX.X, ALU.max,
                                            apply_absolute_value=True)
                # q_scale = exp2(ceil(log2(max(amax,1e-12)/448)))
                zz = Pa.tile([TPC, INH], f32)
                nc.vector.tensor_scalar(zz[:], qscale[:], 1e-12, 1.0 / FP8_MAX, op0=ALU.max, op1=ALU.mult)
                man = Pa.tile([TPC, INH], mybir.dt.uint32)
                nc.vector.tensor_scalar(man[:], zz[:].bitcast(mybir.dt.uint32), 0x007FFFFF, None, op0=ALU.bitwise_and)
                exb = Pa.tile([TPC, INH], mybir.dt.uint32)
                nc.vector.tensor_scalar(exb[:], zz[:].bitcast(mybir.dt.uint32), 0xFF800000, None, op0=ALU.bitwise_and)
                nc.vector.tensor_scalar(man[:], man[:], 0, None, op0=ALU.not_equal)
                nc.vector.tensor_scalar(man[:], man[:], 23, None, op0=ALU.logical_shift_left)
                nc.vector.tensor_tensor(exb[:], exb[:], man[:], op=ALU.add)
                nc.vector.tensor_scalar(qscale[:], exb[:].bitcast(f32), IDX_SCALE * (INH ** -0.5), None, op0=ALU.mult)

                iqT = Pa.tile([IHD, INH, TPC], bf16)
                with tc.tile_pool(name="ps_iqt", bufs=2, space="PSUM") as Pp:
                    for h in range(INH):
                        ti = Pp.tile([IHD, TPC], bf16, name="ti", tag="tp")
                        nc.tensor.transpose(ti[:], iq_bf[:, h, :], idb[:])
                        nc.scalar.copy(iqT[:, h, :], ti[:])

                # wts = (hn @ Wip + b) * qscale_scaled
                wts = Pa.tile([TPC, INH], f32)
                with tc.tile_pool(name="ps_wp", bufs=1, space="PSUM") as Pp:
                    wip_sb = Pa.tile([128, KB, INH], bf16)
                    nc.sync.dma_start(wip_sb[:], WIP[:].rearrange("(k p) n -> p k n", p=128))
                    wp_ps = Pp.tile([TPC, INH], f32)
                    for k in range(KB):
                        nc.tensor.matmul(wp_ps[:], hnT[:, k, :], wip_sb[:, k, :],
                                         start=(k == 0), stop=(k == KB - 1))
                    nc.vector.tensor_add(wts[:], wp_ps[:], wpb_bc[:])
                    nc.vector.tensor_tensor(wts[:], wts[:], qscale[:], op=ALU.mult)

                # ---- gathered tensors -> SBUF (global token order) ----
                kvcT = Pa.tile([128, 4, T], bf16)
                kpeT_all = Pa.tile([DR, T], bf16)
                iknT_all = Pa.tile([IHD, T], bf16)
                for cc in range(NCORES):
                    for hh in range(2):
                        g0 = 64 * cc if hh == 0 else T - 64 * cc - 64
                        src = cc1_out[:][cc]
                        nc.sync.dma_start(
                            kvcT[:, :, g0:g0 + 64],
                            src[:KL, hh * 64:hh * 64 + 64].rearrange("(k p) t -> p k t", p=128))
                        nc.sync.dma_start(kpeT_all[:, g0:g0 + 64],
                                          src[KL:KL + DR, hh * 64:hh * 64 + 64])
                        nc.sync.dma_start(iknT_all[:, g0:g0 + 64],
                                          src[KL + DR:, hh * 64:hh * 64 + 64])
                # V for all tokens from gathered latent -> DRAM scratch
                v_dram = Pd.tile([NCORES, 128, NH * DV], bf16)
                wv_sb = Pa.tile([128, 4, NH * DV], bf16)
                for k in range(4):
                    nc.sync.dma_start(wv_sb[:, k, :], WV[:].rearrange("(k p) n -> p k n", p=128)[:, k, :])
                with tc.tile_pool(name="ps_vall", bufs=2, space="PSUM") as Pp:
                    for tch in range(NCORES):
                        v_ps = Pp.tile([128, NH * DV], f32, name="v_ps", tag="vps")
                        for k in range(4):
                            for j in range(4):
                                nc.tensor.matmul(v_ps[:, j * 512:(j + 1) * 512],
                                                 kvcT[:, k, tch * 128:(tch + 1) * 128],
                                                 wv_sb[:, k, j * 512:(j + 1) * 512],
                                                 start=(k == 0), stop=(k == 3))
                        v_sb = Pa.tile([128, NH * DV], bf16, name="v_sb", tag="vsb", bufs=2)
                        nc.vector.tensor_copy(v_sb[:], v_ps[:])
                        nc.sync.dma_start(v_dram[:][tch], v_sb[:])

                # ---- indexer scores + topk threshold + mask ----
                s_acc = Pa.tile([TPC, T], f32)
                with tc.tile_pool(name="ps_s", bufs=2, space="PSUM") as Pp:
                    for h in range(INH):
                        s_ps = Pp.tile([TPC, T], f32, name="s_ps", tag="sps")
                        for j in range(2):
                            nc.tensor.matmul(s_ps[:, j * 512:(j + 1) * 512], iqT[:, h, :],
                                             iknT_all[:, j * 512:(j + 1) * 512],
                                             start=True, stop=True)
                        if h == 0:
                            nc.vector.tensor_scalar(s_acc[:], s_ps[:], 0.0, wts[:, 0:1],
                                                    op0=ALU.max, op1=ALU.mult)
                        else:
                            tmp_h = Pa.tile([TPC, T], f32, name="tmp_h", tag="tmph")
                            nc.vector.tensor_scalar(tmp_h[:], s_ps[:], 0.0, wts[:, h:h + 1],
                                                    op0=ALU.max, op1=ALU.mult)
                            nc.vector.tensor_add(s_acc[:], s_acc[:], tmp_h[:])
                # causal additive mask; scan copy
                cadd = Pa.tile([TPC, T], f32, tag="maddt")
                nc.vector.tensor_scalar(cadd[:], caus[:], 1.0, -NEG, op0=ALU.subtract, op1=ALU.mult)
                nc.vector.tensor_add(s_acc[:], s_acc[:], cadd[:])
                scr = Pa.tile([TPC, T], f32, tag="scrt")
                nc.vector.tensor_copy(scr[:], s_acc[:])
                m8 = Pa.tile([TPC, 8], f32)
                for it in range(TOPK // 8):
                    nc.vector.max(m8[:], scr[:])
                    nc.vector.match_replace(scr[:], m8[:], scr[:], -3e38)
                # total01 = (s_acc >= thresh) * caus ; mask_add = (total01-1)*1e30
                mask01 = Pa.tile([TPC, T], f32, tag="scrt")
                nc.vector.tensor_scalar(mask01[:], s_acc[:], m8[:, 7:8], None, op0=ALU.is_ge)
                nc.vector.tensor_tensor(mask01[:], mask01[:], caus[:], op=ALU.mult)
                madd = Pa.tile([TPC, T], f32, tag="maddt")
                nc.vector.tensor_scalar(madd[:], mask01[:], 1.0, -NEG, op0=ALU.subtract, op1=ALU.mult)
                madd_bf = Pa.tile([TPC, T], bf16)
                nc.vector.tensor_copy(madd_bf[:], madd[:])

                # ---- MLA attention (K^T built on demand per head) ----
                oT = Pa.tile([DV, NH, TPC], bf16)
                with tc.tile_pool(name="ps_att", bufs=1, space="PSUM") as Pp:
                    for h in range(NH):
                        wkn_h = Pw.tile([128, 4, DN], bf16, name="wkn_h", tag="wknh", bufs=3)
                        nc.sync.dma_start(
                            wkn_h[:],
                            WKN[:, h * DN:(h + 1) * DN].rearrange("(k p) n -> p k n", p=128))
                        kt_ps = Pp.tile([DN, T], f32, name="kt_ps", tag="ktp")
                        for j in range(2):
                            for k in range(4):
                                nc.tensor.matmul(kt_ps[:, j * 512:(j + 1) * 512],
                                                 wkn_h[:, k, :],
                                                 kvcT[:, k, j * 512:(j + 1) * 512],
                                                 start=(k == 0), stop=(k == 3))
                        kt_h = Pa.tile([DN, T], bf16, name="kt_h", tag="kth", bufs=3)
                        nc.vector.tensor_copy(kt_h[:], kt_ps[:])
                        v_h = Pa.tile([128, NCORES, DV], bf16, name="v_h", tag="vh", bufs=3)
                        nc.sync.dma_start(v_h[:], v_dram[:].rearrange("c p d -> p c d")[:, :, h * DV:(h + 1) * DV])
                        a_ps = Pp.tile([TPC, T], f32, name="a_ps", tag="sps", bufs=3)
                        for j in range(2):
                            nc.tensor.matmul(a_ps[:, j * 512:(j + 1) * 512], qtnT[:, h, :],
                                             kt_h[:, j * 512:(j + 1) * 512],
                                             start=True, stop=False)
                            nc.tensor.matmul(a_ps[:, j * 512:(j + 1) * 512], qtpT[:, h, :],
                                             kpeT_all[:, j * 512:(j + 1) * 512],
                                             start=False, stop=False)
                            # += mask via identity matmul (keeps masking on PE)
                            nc.tensor.matmul(a_ps[:, j * 512:(j + 1) * 512], idb[:],
                                             madd_bf[:, j * 512:(j + 1) * 512],
                                             start=False, stop=True)
                        pex = Pa.tile([TPC, T], bf16, name="pex")
                        rs = Pa.tile([TPC, 1], f32, name="rs")
                        nc.scalar.activation(pex[:], a_ps[:], AF.Exp, accum_out=rs[:])
                        nc.vector.reciprocal(rs[:], rs[:])
                        pb = Pa.tile([TPC, T], bf16, name="pb")
                        nc.vector.tensor_scalar(pb[:], pex[:], rs[:], None, op0=ALU.mult)
                        # transpose P in 8 chunks; accumulate O^T
                        o_ps = Pp.tile([DV, TPC], f32, name="o_ps", tag="ops")
                        for s in range(8):
                            pt = Pp.tile([128, TPC], bf16, name="pt", tag="tp")
                            nc.tensor.transpose(pt[:], pb[:, s * 128:(s + 1) * 128], idb[:])
                            pts = Pa.tile([128, TPC], bf16, name="pts", tag="pts")
                            nc.vector.tensor_copy(pts[:], pt[:])
                            nc.tensor.matmul(o_ps[:], v_h[:, s, :], pts[:],
                                             start=(s == 0), stop=(s == 7))
                        nc.vector.tensor_copy(oT[:, h, :], o_ps[:])

                # ---- o_proj + residual ----
                x_own = Pa.tile([TPC, H], f32)
                with tc.tile_pool(name="ps_op", bufs=1, space="PSUM") as Pp:
                    d_ps = Pp.tile([TPC, H], f32)
                    for h in range(NH):
                        wo_k = Pw.tile([128, H], bf16, name="wstream4", tag="wstream")
                        nc.sync.dma_start(wo_k[:], WO[:].rearrange("(k p) n -> p k n", p=128)[:, h, :])
                        for j in range(4):
                            nc.tensor.matmul(d_ps[:, j * 512:(j + 1) * 512], oT[:, h, :],
                                             wo_k[:, j * 512:(j + 1) * 512],
                                             start=(h == 0), stop=(h == NH - 1))
                    nc.vector.tensor_tensor(x_own[:], d_ps[:], xo[:], op=ALU.add)

                # ---- gate logits (f32) + rw + h2 ----
                xT_own = Pa.tile([128, KB, TPC], f32)
                with tc.tile_pool(name="ps_xt", bufs=2, space="PSUM") as Pp:
                    for k in range(KB):
                        tx = Pp.tile([128, TPC], f32, name="tx", tag="tpf")
                        nc.tensor.transpose(tx[:], x_own[:, k * 128:(k + 1) * 128], idf[:])
                        nc.scalar.copy(xT_own[:, k, :], tx[:])
                sq2 = Pa.tile([TPC, H], f32, name="sq2a", tag="sq2")
                ss2 = Pa.tile([TPC, 1], f32)
                nc.scalar.activation(sq2[:], x_own[:], AF.Square, accum_out=ss2[:])
                r2 = Pa.tile([TPC, 1], f32)
                nc.scalar.activation(r2[:], ss2[:], AF.Sqrt, bias=eps_b[:], scale=1.0 / H)
                nc.vector.reciprocal(r2[:], r2[:])
                lg = Pa.tile([TPC, NE], f32)
                with tc.tile_pool(name="ps_g", bufs=1, space="PSUM") as Pp:
                    l_ps = Pp.tile([TPC, NE], f32)
                    for k in range(KB):
                        nc.tensor.matmul(l_ps[:], xT_own[:, k, :], wg_sb[:, k, :],
                                         start=(k == 0), stop=(k == KB - 1))
                    nc.scalar.activation(lg[:], l_ps[:], AF.Copy, scale=r2[:])
                gm8 = Pa.tile([TPC, 8], f32)
                nc.vector.max(gm8[:], lg[:])
                negm0 = Pa.tile([TPC, 1], f32)
                nc.vector.tensor_scalar(negm0[:], gm8[:, 0:1], -1.0, None, op0=ALU.mult)
                el = Pa.tile([TPC, NE], f32)
                nc.scalar.activation(el[:], lg[:], AF.Exp, bias=negm0[:])
                dn1 = Pa.tile([TPC, 1], f32)
                nc.vector.tensor_tensor(dn1[:], gm8[:, 1:2], gm8[:, 0:1], op=ALU.subtract)
                nc.scalar.activation(dn1[:], dn1[:], AF.Exp)
                nc.vector.tensor_scalar(dn1[:], dn1[:], 1.0, None, op0=ALU.add)
                nc.vector.reciprocal(dn1[:], dn1[:])
                sel = Pa.tile([TPC, NE], f32)
                nc.vector.tensor_scalar(sel[:], lg[:], gm8[:, 1:2], None, op0=ALU.is_ge)
                rw = Pa.tile([TPC, NE], f32)
                nc.vector.scalar_tensor_tensor(rw[:], el[:], dn1[:], sel[:],
                                               op0=ALU.mult, op1=ALU.mult)
                h2_own = Pa.tile([TPC, H], bf16)
                nc.vector.tensor_scalar(h2_own[:], x_own[:], r2[:], None, op0=ALU.mult)

                # ---- CC#2 ----
                nc.sync.dma_start(cch_in[:], h2_own[:])
                nc.sync.dma_start(ccr_in[:], rw[:])
                nc.gpsimd.collective_compute("AllGather", ALU.bypass, replica_groups=RG,
                                             ins=[cch_in[:].opt()], outs=[cch_out[:].opt()])
                nc.gpsimd.collective_compute("AllGather", ALU.bypass, replica_groups=RG,
                                             ins=[ccr_in[:].opt()], outs=[ccr_out[:].opt()])

                # ---- shared expert on own tokens (overlaps CC#2) ----
                h2T_own = Pa.tile([128, KB, TPC], bf16)
                r2row = Pa.tile([1, TPC], f32)
                r2bc = Pa.tile([128, TPC], f32)
                with tc.tile_pool(name="ps_r2", bufs=1, space="PSUM") as Pp:
                    r2p = Pp.tile([1, TPC], f32)
                    nc.tensor.transpose(r2p[:], r2[:], idf[:])
                    nc.scalar.copy(r2row[:], r2p[:])
                nc.gpsimd.partition_broadcast(r2bc[:], r2row[:])
                for k in range(KB):
                    nc.vector.tensor_tensor(h2T_own[:, k, :], xT_own[:, k, :], r2bc[:], op=ALU.mult)
                ss_own = Pa.tile([TPC, SI], bf16)
                with tc.tile_pool(name="ps_sh", bufs=1, space="PSUM") as Pp:
                    gs_ps = Pp.tile([TPC, SI], f32, name="gs_ps")
                    us_ps = Pp.tile([TPC, SI], f32, name="us_ps")
                    for k in range(KB):
                        wsg_k = Pw.tile([128, SI], bf16, name="wsg_k", tag="wstream")
                        nc.sync.dma_start(wsg_k[:], WSG[:].rearrange("(k p) n -> p k n", p=128)[:, k, :])
                        wsu_k = Pw.tile([128, SI], bf16, name="wsu_k", tag="wstream")
                        nc.sync.dma_start(wsu_k[:], WSU[:].rearrange("(k p) n -> p k n", p=128)[:, k, :])
                        for j in range(2):
                            nc.tensor.matmul(gs_ps[:, j * 512:(j + 1) * 512], h2T_own[:, k, :],
                                             wsg_k[:, j * 512:(j + 1) * 512],
                                             start=(k == 0), stop=(k == KB - 1))
                            nc.tensor.matmul(us_ps[:, j * 512:(j + 1) * 512], h2T_own[:, k, :],
                                             wsu_k[:, j * 512:(j + 1) * 512],
                                             start=(k == 0), stop=(k == KB - 1))
                    sgo = Pa.tile([TPC, SI], f32, name="sgo", tag="sq2")
                    nc.scalar.activation(sgo[:], gs_ps[:], AF.Silu)
                    nc.vector.tensor_tensor(ss_own[:], sgo[:], us_ps[:], op=ALU.mult)
                ssT = Pa.tile([128, 8, TPC], bf16)
                with tc.tile_pool(name="ps_st", bufs=2, space="PSUM") as Pp:
                    for m in range(8):
                        tss = Pp.tile([128, TPC], bf16, name="tss", tag="tp")
                        nc.tensor.transpose(tss[:], ss_own[:, m * 128:(m + 1) * 128], idb[:])
                        nc.vector.tensor_copy(ssT[:, m, :], tss[:])
                with tc.tile_pool(name="ps_sd", bufs=1, space="PSUM") as Pp:
                    sh_ps = Pp.tile([TPC, H], f32)
                    for m in range(8):
                        wsd_m = Pw.tile([128, H], bf16, name="wsd_m", tag="wstream")
                        nc.sync.dma_start(wsd_m[:], WSD[:].rearrange("(k p) n -> p k n", p=128)[:, m, :])
                        for j in range(4):
                            nc.tensor.matmul(sh_ps[:, j * 512:(j + 1) * 512], ssT[:, m, :],
                                             wsd_m[:, j * 512:(j + 1) * 512],
                                             start=(m == 0), stop=(m == 7))
                    outx = Pa.tile([TPC, H], f32, name="outx", tag="sq2")
                    nc.vector.tensor_tensor(outx[:], sh_ps[:], x_own[:], op=ALU.add)
                nc.sync.dma_start(OUT_X[:], outx[:])

            # =================== MoE phase ===================
            with tc.tile_pool(name="moe", bufs=1) as Pm:
                h2T = Pm.tile([128, KB, T], bf16)
                h2flat = cch_out[:].rearrange("c p n -> (c p) n")
                for tb in range(4):
                    nc.sync.dma_start_transpose(h2T[:, :, tb * 256:(tb + 1) * 256],
                                                h2flat[tb * 256:(tb + 1) * 256, :])
                # rw row for own expert -> broadcast
                rwT = Pm.tile([NE, T], f32)
                nc.sync.dma_start(rwT[:], ccr_out[:].rearrange("c p n -> n (c p)"))
                rw_bc = Pm.tile([128, T], f32)
                with tc.tile_pool(name="ps_rw", bufs=1, space="PSUM") as Pp:
                    rr_ps = Pp.tile([1, T], f32)
                    for j in range(2):
                        nc.tensor.matmul(rr_ps[:, j * 512:(j + 1) * 512], oh[:],
                                         rwT[:, j * 512:(j + 1) * 512], start=True, stop=True)
                    rw_row = Pm.tile([1, T], f32)
                    nc.scalar.copy(rw_row[:], rr_ps[:])
                nc.gpsimd.partition_broadcast(rw_bc[:], rw_row[:])

                weg = Pm.tile([128, KB, MI], bf16)
                weu = Pm.tile([128, KB, MI], bf16)
                for k in range(KB):
                    nc.sync.dma_start(weg[:, k, :], WEG[:].rearrange("(k p) n -> p k n", p=128)[:, k, :])
                    nc.sync.dma_start(weu[:, k, :], WEU[:].rearrange("(k p) n -> p k n", p=128)[:, k, :])
                su = Pm.tile([128, NE, T], bf16)       # scaled silu(g)*u*rw per mi-chunk
                with tc.tile_pool(name="ps_moe", bufs=2, space="PSUM") as Pp:
                    for m in range(8):
                        g_ps = Pp.tile([128, T], f32, name="g_ps", tag="gps")
                        u_ps = Pp.tile([128, T], f32, name="u_ps", tag="ups")
                        for k in range(KB):
                            for j in range(2):
                                nc.tensor.matmul(g_ps[:, j * 512:(j + 1) * 512],
                                                 weg[:, k, m * 128:(m + 1) * 128],
                                                 h2T[:, k, j * 512:(j + 1) * 512],
                                                 start=(k == 0), stop=(k == KB - 1))
                                nc.tensor.matmul(u_ps[:, j * 512:(j + 1) * 512],
                                                 weu[:, k, m * 128:(m + 1) * 128],
                                                 h2T[:, k, j * 512:(j + 1) * 512],
                                                 start=(k == 0), stop=(k == KB - 1))
                        sg = Pm.tile([128, T], f32, name="sg", tag="sgs")
                        nc.scalar.activation(sg[:], g_ps[:], AF.Silu)
                        nc.vector.tensor_tensor(sg[:], sg[:], u_ps[:], op=ALU.mult)
                        nc.vector.tensor_tensor(su[:, m, :], sg[:], rw_bc[:], op=ALU.mult)

                wed = Pm.tile([128, NE, H], bf16)
                for k in range(NE):
                    nc.sync.dma_start(wed[:, k, :], WED[:].rearrange("(k p) n -> p k n", p=128)[:, k, :])
                with tc.tile_pool(name="ps_dn", bufs=2, space="PSUM") as Pp:
                    for tch in range(8):
                        dn_ps = Pp.tile([128, H], f32, name="dn_ps", tag="dnp")
                        for m in range(8):
                            for j in range(4):
                                nc.tensor.matmul(dn_ps[:, j * 512:(j + 1) * 512],
                                                 su[:, m, tch * 128:(tch + 1) * 128],
                                                 wed[:, m, j * 512:(j + 1) * 512],
                                                 start=(m == 0), stop=(m == 7))
                        ob = Pm.tile([128, H], f32, name="ob", tag="obs")
                        nc.scalar.copy(ob[:], dn_ps[:])
                        nc.sync.dma_start(OUT_P[:].rearrange("(c p) n -> c p n", p=128)[tch], ob[:])

    nc.compile()
    return nc


_NC = None


def kernel(**inputs):
    global _NC
    inp = {k: np.asarray(v) for k, v in inputs.items()}
    pos = inp["positions"].astype(np.int64)
    x = inp["hidden_states"].astype(np.float32)

    # ---- fold layernorm weights into downstream mats (host prep) ----
    iw = inp["input_ln_w"].astype(np.float32)
    qw = inp["q_a_ln_w"].astype(np.float32)
    kw = inp["kv_a_ln_w"].astype(np.float32)
    pw = inp["post_ln_w"].astype(np.float32)
    Wa = (iw[:, None] * inp["W_qkv_a"]).astype(BF)
    Wik = (iw[:, None] * inp["idx_wk"]).astype(BF)
    Wip = (iw[:, None] * inp["idx_wp_w"]).astype(BF)
    Wqb = (qw[:, None] * inp["W_q_b"]).astype(BF)
    Wiq = (qw[:, None] * inp["idx_wq_b"]).astype(BF)
    Wkvb = (kw[:, None] * inp["W_kv_b"]).astype(np.float32).reshape(KL, NH, DN + DV)
    Wkn = np.ascontiguousarray(Wkvb[:, :, :DN].reshape(KL, NH * DN)).astype(BF)
    Wv = np.ascontiguousarray(Wkvb[:, :, DN:].reshape(KL, NH * DV)).astype(BF)
    Wo = inp["W_o"].astype(BF)
    Wg = (pw[:, None] * inp["W_gate"]).astype(np.float32)
    Weg = (pw[None, :, None] * inp["We_gate"]).astype(BF)
    Weu = (pw[None, :, None] * inp["We_up"]).astype(BF)
    Wed = inp["We_down"].astype(BF)
    Wsg = (pw[:, None] * inp["Ws_gate"]).astype(BF)
    Wsu = (pw[:, None] * inp["Ws_up"]).astype(BF)
    Wsd = inp["Ws_down"].astype(BF)

    inv = 1.0 / (BASE ** (np.arange(0, DR, 2, dtype=np.float32) / DR))
    ang = pos.astype(np.float32)[:, None] * inv           # [T, 32]
    cs_a, sn_a = np.cos(ang), np.sin(ang)

    SIS = SI // NCORES
    in_maps = []
    for c in range(NCORES):
        rows = own_rows(c)
        posn = pos[rows]
        caus = (posn[:, None] >= pos[None, :]).astype(np.float32)
        cs = cs_a[rows]; sn = sn_a[rows]
        oh = np.zeros((8, 1), np.float32); oh[c, 0] = 1.0
        in_maps.append({
            "XO": np.ascontiguousarray(x[rows]),
            "CAUS": np.ascontiguousarray(caus),
            "CSQ": np.ascontiguousarray(np.tile(cs * SCALE, (1, NH)).astype(np.float32)),
            "SNQ": np.ascontiguousarray(np.tile(sn * SCALE, (1, NH)).astype(np.float32)),
            "CSR": np.ascontiguousarray(np.tile(cs, (1, NH)).astype(np.float32)),
            "SNR": np.ascontiguousarray(np.tile(sn, (1, NH)).astype(np.float32)),
            "OH": oh,
            "KNW": inp["idx_kn_w"].astype(np.float32).reshape(1, IHD),
            "KNB": inp["idx_kn_b"].astype(np.float32).reshape(1, IHD),
            "WPB": inp["idx_wp_b"].astype(np.float32).reshape(1, INH),
            "WA": Wa, "WQB": Wqb, "WIQ": Wiq, "WIK": Wik, "WIP": Wip,
            "WKN": Wkn, "WV": Wv, "WO": Wo, "WG": Wg,
            "WEG": np.ascontiguousarray(Weg[c]),
            "WEU": np.ascontiguousarray(Weu[c]),
            "WED": np.ascontiguousarray(Wed[c]),
            "WSG": Wsg, "WSU": Wsu, "WSD": Wsd,
        })

    if _NC is None:
        _NC = build()
    res = run_bass_kernel_spmd(_NC, in_maps, core_ids=list(range(NCORES)))

    out_perm = np.zeros((T, H), np.float64)
    for c in range(NCORES):
        out_perm += res.results[c]["OUT_P"].astype(np.float64)
    for c in range(NCORES):
        out_perm[c * TPC:(c + 1) * TPC] += res.results[c]["OUT_X"].astype(np.float64)
    final = np.empty((T, H), np.float32)
    final[PERM] = out_perm.astype(np.float32)
    return final
